# revision 79
# baseline (speedup 1.0000x reference)
"""Trainium2 Bass kernel for nn_BernoulliDecompAttModel (decomposable attention NLI model).

Contract: kernel(**inputs) takes the FULL unsharded inputs (as produced by
setup_inputs()) and returns the FULL [64, 3] float32 output. Internally the
batch (64) is sharded 8-ways across 8 NeuronCores (pure data parallel, all
weights replicated); each core runs an identical Bass/Tile program on its 8
batch items.

Layout conventions inside the device program (per core):
  - Activations are kept "feature-major": [128 partitions = feature chunk,
    K/128 chunks, token axis on free dim]. Matmul contracts over partitions,
    so feature-major activations feed matmuls directly with weights stored
    natural [Kin (partitions x chunks), Nout].
  - prem and hypo (256 tokens each) are concatenated on the token axis for
    every shared-weight MLP -> N=512 matmuls.
  - All matmul operands are float32r (fp32 storage, 1 cycle/column on the PE
    vs 4 for plain fp32; ~2^-13 effective multiply rounding).
  - Attention: scores accumulate in PSUM; key masks are injected by a K=1
    matmul ones[1,128] (x) madd[1,256] accumulated into the same PSUM bank.
    The relative-distance bias matrix (incl. the score-diagonal -1e9) and the
    pad masks are precomputed on the host and DMA'd like weights.

Performance notes (964us -> ~515us on 8 trn2 cores):
  - PSUM tags are multi-buffered (mm512 x4 banks, attn x2, attnr x2) so the
    PE never waits for Scalar/Vector to drain an accumulation bank; keeping
    the PE continuously busy also lets it ramp from the 1.2 GHz mid p-state
    toward 2.4 GHz (the ramp needs ~3us of uninterrupted occupancy, so every
    avoided bubble also buys clock speed on the instructions that follow).
  - Token index DMAs + embedding gathers for all items are issued early and
    on separate trigger queues from the weight loads (weights on SP/sync,
    per-item loads on Activation/scalar, both in first-use order), so item-0
    matmul work starts ~15us in instead of ~70us.
  - cmpin/mid/qpq SBUF tags are double-buffered so item b+1's transposes and
    MLP fill the PE while item b is in its softmax chains; wg1/wg2 are loaded
    late into retired cmpin slots to make the footprint fit (224KB SBUF).
  - Four 128x128 PE transposes fill one 2KB PSUM bank and drain with a
    single strided DVE copy; softmaxes read the drained SBUF copy so the
    attention banks free as early as possible. The compare-L1 activations
    and wc2 run in bf16 (gpsimd casting DMA) to fund double-buffered
    embedding-gather tiles.
  - The per-item masked token sums collect in an SBUF [40,512] tile (via tiny
    SBUF->SBUF DMAs) and are PE-transposed for the aggregate MLP, avoiding a
    DRAM round-trip in the tail.
"""

import numpy as np
import os

B, L, V, E, D, OUT = 64, 256, 50000, 512, 512, 3
NCORES = 8
BL = B // NCORES            # batch items per core
MAX_DIST = 11
MASK_VAL = -30000.0         # padded-key additive mask (exp() underflows to 0)
DIAG_VAL = -1e9             # self-attention diagonal

_PROG_CACHE = {}


def _build_program(debug_taps=()):
    import concourse.bass as bass
    import concourse.bacc as bacc
    import concourse.mybir as mybir
    from concourse.tile import TileContext
    from concourse.masks import make_identity

    dt = mybir.dt
    f32, f32r, i32 = dt.float32, dt.float32r, dt.int32
    bf16 = dt.bfloat16
    AF = mybir.ActivationFunctionType
    ALU = mybir.AluOpType
    AX = mybir.AxisListType

    nc = bacc.Bacc("TRN2", target_bir_lowering=False, debug=True)

    # ---------------- DRAM I/O ----------------
    tok = nc.dram_tensor("tok", [2, BL, L], i32, kind="ExternalInput")
    emb = nc.dram_tensor("emb", [V, E], f32, kind="ExternalInput")
    # host-precomputed: bias_m[ic, p, j] = rel-bias[ic*128+p, j] (+ diag -1e9)
    bias_md = nc.dram_tensor("bias_m", [2, 128, 256], f32, kind="ExternalInput")
    # host-precomputed masks: mf[b, s, tc, p] = (tok != 0); md[b, s, j] = -3e4*(tok == 0)
    mf_d = nc.dram_tensor("mf", [BL, 2, 2, 128], f32r, kind="ExternalInput")
    md_d = nc.dram_tensor("md", [BL, 2, L], f32r, kind="ExternalInput")
    w_s1 = nc.dram_tensor("w_s1", [E, D], f32r, kind="ExternalInput")
    w_s2 = nc.dram_tensor("w_s2", [D, D], f32r, kind="ExternalInput")
    w_a1 = nc.dram_tensor("w_a1", [2 * E, D], f32r, kind="ExternalInput")
    w_a2 = nc.dram_tensor("w_a2", [D, D], f32r, kind="ExternalInput")
    w_c1 = nc.dram_tensor("w_c1", [4 * E, D], f32r, kind="ExternalInput")
    w_c2 = nc.dram_tensor("w_c2", [D, D], f32, kind="ExternalInput")
    w_g1 = nc.dram_tensor("w_g1", [2 * D, D], f32r, kind="ExternalInput")
    w_g2 = nc.dram_tensor("w_g2", [D, D], f32r, kind="ExternalInput")
    w_o = nc.dram_tensor("w_o", [D, OUT], f32r, kind="ExternalInput")
    bias_names = ["b_s1", "b_s2", "b_a1", "b_a2", "b_c1", "b_c2", "b_g1", "b_g2"]
    bdram = {n: nc.dram_tensor(n, [D], f32, kind="ExternalInput") for n in bias_names}

    out_d = nc.dram_tensor("out", [BL, OUT], f32, kind="ExternalOutput")

    dbg = {}
    for name, shape in debug_taps:
        dbg[name] = nc.dram_tensor(name, shape, f32, kind="ExternalOutput")

    with TileContext(nc) as tc:
        const = tc.alloc_tile_pool(name="const", bufs=1)
        work = tc.alloc_tile_pool(name="work", bufs=1)
        ps = tc.alloc_tile_pool(name="ps", bufs=1, space="PSUM")

        def mm512_ps(name, shape=(128, 512)):
            return ps.tile(list(shape), f32, space="PSUM", tag="mm512", bufs=4,
                           name=name)

        def attn_ps(name):
            return ps.tile([128, 256], f32, space="PSUM", tag="attn", bufs=2,
                           name=name)

        def attnr_ps(name, shape=(128, 256)):
            return ps.tile(list(shape), f32r, space="PSUM", tag="attnr", bufs=2,
                           name=name)

        # ---------------- small constants (no big DMA) ----------------
        ones32 = const.tile([1, 128], f32, name="ones32")
        nc.vector.memset(ones32[:], 1.0)
        onesr = const.tile([1, 128], f32r, name="onesr")
        nc.vector.tensor_copy(onesr[:], ones32[:])
        ones = onesr[:]

        ident32 = const.tile([128, 128], f32, name="ident32")
        make_identity(nc, ident32[:])
        identr_t = const.tile([128, 128], f32r, name="identr_t")
        nc.vector.tensor_copy(identr_t[:], ident32[:])
        identr = identr_t[:]

        # Weights all on the sync queue in first-use order; per-item DMAs on
        # the scalar queue. Self-MLP weights first: item 0 needs them ~15us in.
        def load_w(dram, K, N, name, eng):
            t = const.tile([128, K // 128, N], f32r, name=name)
            eng.dma_start(t[:], bass.AP(dram, 0, [[N, 128], [128 * N, K // 128], [1, N]]))
            return t

        bsb = {}

        def load_b(n):
            t = const.tile([128, 4], f32, name=f"sb_{n}")
            nc.sync.dma_start(t[:], bass.AP(bdram[n], 0, [[1, 128], [128, 4]]))
            bsb[n] = t

        ws1 = load_w(w_s1, E, D, "ws1", nc.sync)
        ws2 = load_w(w_s2, D, D, "ws2", nc.sync)
        load_b("b_s1")
        load_b("b_s2")
        # relative-distance bias matrix (host-precomputed, incl. diag -1e9)
        bias_sb = const.tile([128, 2, 256], f32, name="bias_sb")
        nc.sync.dma_start(bias_sb[:], bass.AP(bias_md, 0, [[256, 128], [128 * 256, 2], [1, 256]]))

        # ---------------- per-item indices / masks / gathers ----------------
        # Issued before the remaining weight loads so early item PE work can
        # start while weights stream in.
        nitems = int(os.environ.get('KITEMS', BL))
        STAGE = int(os.environ.get('KSTAGE', 99))

        it_all, maskf_all, madd_all, xembf_all = {}, {}, {}, {}
        for b in range(nitems):
            # items 0-1 trigger on scalar (sync is busy issuing weights then);
            # later items trigger on sync, keeping the scalar queue pure-RELU
            # in steady state
            teng = nc.scalar if b < 2 else nc.sync
            it = work.tile([128, 2, 2], i32, tag="it", bufs=4, name=f"it{b}")
            for s in range(2):
                teng.dma_start(it[:, s, :], bass.AP(tok, b * L + s * BL * L, [[1, 128], [128, 2]]))
            nc.vector.tensor_scalar(it[:], it[:], 12, None, op0=ALU.arith_shift_right)
            maskf = work.tile([128, 2, 2], f32r, tag="maskf", bufs=8, name=f"maskf{b}")
            teng.dma_start(maskf[:].rearrange("p a b -> p (a b)"),
                           bass.AP(mf_d, b * 512, [[1, 128], [128, 4]]))
            madd = work.tile([1, 2, L], f32r, tag="madd", bufs=2, name=f"madd{b}")
            teng.dma_start(madd[:], bass.AP(md_d, b * 2 * L, [[0, 1], [L, 2], [1, L]]))

            # embedding gather (token-major), straight into f32r tiles (DMA
            # outputs need no explicit f32r rounding op)
            xembf = [work.tile([128, E], f32r, tag=f"xembf{i}", bufs=2, name=f"xembf{b}_{i}")
                     for i in range(4)]
            for s in range(2):
                for tcn in range(2):
                    i = s * 2 + tcn
                    nc.gpsimd.indirect_dma_start(
                        out=xembf[i][:, :], out_offset=None, in_=emb.ap().bitcast(f32r),
                        in_offset=bass.IndirectOffsetOnAxis(ap=it[:, s, tcn:tcn + 1], axis=0))
            it_all[b], maskf_all[b], madd_all[b], xembf_all[b] = it, maskf, madd, xembf

        if "dbg_it" in dbg:
            itf = work.tile([128, 4], f32, tag="dbgitf", name="dbgitf")
            nc.vector.tensor_copy(itf[:], it_all[0][:].rearrange("p a b -> p (a b)"))
            nc.sync.dma_start(dbg["dbg_it"].ap(), itf[:])
            embrow = work.tile([2, 512], f32, tag="dbgemb", name="dbgemb")
            nc.sync.dma_start(embrow[0:1, :], bass.AP(emb, 7402 * 512, [[0, 1], [1, 512]]))
            nc.sync.dma_start(embrow[1:2, :], bass.AP(emb, 8192 * 512, [[0, 1], [1, 512]]))
            nc.sync.dma_start(dbg["dbg_embrow"].ap(), embrow[:])
        if "dbg_xemb" in dbg:
            for i in range(4):
                nc.sync.dma_start(bass.AP(dbg["dbg_xemb"], i * 512, [[4 * 512, 128], [1, 512]]),
                                  xembf_all[0][i][:, :].bitcast(f32))

        # ---------------- remaining weights (first-use order) ----
        wa1 = load_w(w_a1, 2 * E, D, "wa1", nc.sync)
        wa2 = load_w(w_a2, D, D, "wa2", nc.sync)
        load_b("b_a1")
        load_b("b_a2")
        wc1 = load_w(w_c1, 4 * E, D, "wc1", nc.sync)
        wc2 = const.tile([128, 4, D], bf16, name="wc2")
        nc.gpsimd.dma_start(wc2[:], bass.AP(w_c2, 0, [[D, 128], [128 * D, 4], [1, D]]))
        load_b("b_c1")
        load_b("b_c2")
        bc2row = const.tile([1, D], f32r, name="bc2row")
        nc.sync.dma_start(bc2row[:], bass.AP(bdram["b_c2"], 0, [[0, 1], [1, D]]).bitcast(f32r))
        load_b("b_g1")
        load_b("b_g2")
        wo = const.tile([128, 4, 4], f32r, name="wo")
        nc.vector.memset(wo[:].bitcast(f32), 0.0)
        nc.sync.dma_start(wo[:, :, 0:OUT], bass.AP(w_o, 0, [[OUT, 128], [128 * OUT, 4], [1, OUT]]))

        # masked per-item sums collect here; prem rows at partitions 0..7,
        # hypo rows at 32..39 (matmul lhsT base partition must be 0/32/64)
        s16 = work.tile([32 + BL, 512], f32r, tag="s16", name="s16")

        # ---------------- per-item pipeline ----------------
        def softmax_rows(src_ap, dst_ap, tag_suffix):
            """row softmax: src_ap [128,256] (SBUF or PSUM, f32 view) -> dst_ap f32r"""
            negmax = work.tile([128, 1], f32, tag="negmax", bufs=4, name=f"negmax{tag_suffix}")
            nc.vector.reduce_max(negmax[:], src_ap, axis=AX.X, negate=True)
            esum = work.tile([128, 1], f32, tag="esum", bufs=4, name=f"esum{tag_suffix}")
            nc.scalar.activation(dst_ap, src_ap, AF.Exp, bias=negmax[:], scale=1.0,
                                 accum_out=esum[:])
            rec = work.tile([128, 1], f32, tag="rec", bufs=4, name=f"rec{tag_suffix}")
            nc.vector.reciprocal(rec[:], esum[:])
            nc.vector.tensor_scalar(dst_ap, dst_ap, rec[:, 0:1], None, op0=ALU.mult)

        for b in range(nitems):
            it, maskf, madd, xembf = it_all[b], maskf_all[b], madd_all[b], xembf_all[b]
            xemb = [t[:] for t in xembf]

            if STAGE < 2:
                continue
            # ---- x transposes -> cmpin kc 0..3 (feature-major cat, both seqs) ----
            # four 128x128 transposes fill one 2KB PSUM bank -> one strided drain
            cmpin = work.tile([128, 8, 512], f32r, tag="cmpin", bufs=2, name=f"cmpin{b}")
            for s in range(2):
                for tcn in range(2):
                    ptr = attnr_ps(f"xT{b}_{s}{tcn}0", (128, 512))
                    for k in range(4):
                        nc.tensor.matmul(ptr[:, k * 128:(k + 1) * 128],
                                         lhsT=xemb[s * 2 + tcn][:, k * 128:(k + 1) * 128],
                                         rhs=identr, is_transpose=True, start=(k == 0), stop=(k == 3))
                    nc.vector.tensor_copy(
                        cmpin[:, 0:4, s * 256 + tcn * 128:s * 256 + (tcn + 1) * 128],
                        ptr[:].rearrange("p (a b) -> p a b", a=4))

            if STAGE < 3:
                continue
            # ---- self MLP (shared weights, both seqs: N=512) ----
            def mlp_fm(src, src_kcs, w, bias_t, dst, name):
                """feature-major MLP layer: dst[:,nf,:] = relu(w.T @ src + bias)

                The last chunk's drain runs on DVE (bias-add then max-0 in one
                tensor_scalar) so a consumer needing all 4 chunks isn't
                serialized behind Scalar's queue."""
                nkc = len(src_kcs)
                for nf in range(4):
                    pm = mm512_ps(f"{name}_nf{nf}")
                    for i, kc in enumerate(src_kcs):
                        nc.tensor.matmul(pm[:], lhsT=w[:, kc, nf * 128:(nf + 1) * 128],
                                         rhs=src[:, kc, :], start=(i == 0), stop=(i == nkc - 1))
                    nc.scalar.activation(dst[:, nf, :], pm[:], AF.Relu, bias=bias_t[:, nf:nf + 1])

            hmid = work.tile([128, 4, 512], f32r, tag="mid", bufs=2, name=f"h1_{b}")
            mlp_fm(cmpin, range(4), ws1, bsb["b_s1"], hmid, f"sm1_{b}")
            qb = work.tile([128, 4, 512], f32r, tag="qpq", bufs=2, name=f"q_{b}")
            mlp_fm(hmid, range(4), ws2, bsb["b_s2"], qb, f"sm2_{b}")

            if STAGE < 4:
                continue
            # ---- self attention per sequence ----
            att = work.tile([128, 4, 256], f32r, tag="att", name=f"att{b}")
            for s in range(2):
                for ic in range(2):
                    pS = attn_ps(f"S{b}_{s}{ic}")
                    nc.tensor.matmul(pS[:], lhsT=ones, rhs=madd[0:1, s, :],
                                     start=True, stop=False)
                    for kc in range(4):
                        nc.tensor.matmul(pS[:], lhsT=qb[:, kc, s * 256 + ic * 128:s * 256 + (ic + 1) * 128],
                                         rhs=qb[:, kc, s * 256:(s + 1) * 256],
                                         start=False, stop=(kc == 3))
                    sm = work.tile([128, 256], f32, tag="sm", bufs=2, name=f"sm{b}_{s}{ic}")
                    nc.vector.tensor_tensor(sm[:], pS[:], bias_sb[:, ic, :], op=ALU.add)
                    softmax_rows(sm[:], att[:, s * 2 + ic, :], f"_att{b}_{s}{ic}")

            if STAGE < 5:
                continue
            attT = work.tile([128, 4, 256], f32r, tag="attT", name=f"attT{b}")
            for s in range(2):
                ptr = attnr_ps(f"attT{b}_{s}", (128, 512))
                for jc in range(2):
                    for ic in range(2):
                        k = jc * 2 + ic
                        nc.tensor.matmul(ptr[:, k * 128:(k + 1) * 128],
                                         lhsT=att[:, s * 2 + ic, jc * 128:(jc + 1) * 128],
                                         rhs=identr, is_transpose=True, start=(k == 0), stop=(k == 3))
                nc.vector.tensor_copy(attT[:, s * 2:s * 2 + 2, :],
                                      ptr[:].rearrange("p (a b) -> p a b", a=2))

            # ---- ctx feature-major -> cmpin[:, 4+dc, :] ----
            for s in range(2):
                for dc in range(4):
                    pm = attn_ps(f"ctxT{b}_{s}{dc}")
                    for jc in range(2):
                        nc.tensor.matmul(pm[:], lhsT=xemb[s * 2 + jc][:, dc * 128:(dc + 1) * 128],
                                         rhs=attT[:, s * 2 + jc, :], start=(jc == 0), stop=(jc == 1))
                    nc.vector.tensor_copy(cmpin[:, 4 + dc, s * 256:(s + 1) * 256], pm[:])

            if STAGE < 6:
                continue
            # ---- inter MLP (input = cmpin kc 0..7, K=1024) ----
            hmid2 = work.tile([128, 4, 512], f32r, tag="mid", bufs=2, name=f"h2_{b}")
            mlp_fm(cmpin, range(8), wa1, bsb["b_a1"], hmid2, f"im1_{b}")
            qb2 = work.tile([128, 4, 512], f32r, tag="qpq", bufs=2, name=f"q2_{b}")
            mlp_fm(hmid2, range(4), wa2, bsb["b_a2"], qb2, f"im2_{b}")  # qb2 = [pq | hk]

            if STAGE < 7:
                continue
            # ---- inter attention z = pq @ hk^T ----
            zm = work.tile([128, 2, 256], f32r, tag="zm", name=f"zm{b}")
            p2h = work.tile([128, 2, 256], f32r, tag="ph", bufs=2, name=f"p2h{b}")
            for ic in range(2):
                pz = attn_ps(f"z{b}_{ic}")
                nc.tensor.matmul(pz[:], lhsT=ones, rhs=madd[0:1, 1, :], start=True, stop=False)
                for kc in range(4):
                    nc.tensor.matmul(pz[:], lhsT=qb2[:, kc, ic * 128:(ic + 1) * 128],
                                     rhs=qb2[:, kc, 256:512], start=False, stop=(kc == 3))
                nc.vector.tensor_copy(zm[:, ic, :], pz[:])
                softmax_rows(zm[:, ic, :].bitcast(f32), p2h[:, ic, :], f"_p2h{b}_{ic}")

            h2p = work.tile([128, 2, 256], f32r, tag="ph", bufs=2, name=f"h2p{b}")
            for jc in range(2):
                pzT = attnr_ps(f"zT{b}_{jc}")
                nc.tensor.matmul(pzT[:].bitcast(f32), lhsT=ones, rhs=madd[0:1, 0, :].bitcast(f32r),
                                 start=True, stop=False)
                for ic in range(2):
                    nc.tensor.matmul(pzT[:, ic * 128:(ic + 1) * 128],
                                     lhsT=zm[:, ic, jc * 128:(jc + 1) * 128],
                                     rhs=identr, is_transpose=True, start=False, stop=(ic == 1))
                softmax_rows(pzT[:].bitcast(f32), h2p[:, jc, :], f"_h2p{b}_{jc}")

            if STAGE < 8:
                continue
            p2hT = work.tile([128, 2, 256], f32r, tag="phT", bufs=2, name=f"p2hT{b}")
            h2pT = work.tile([128, 2, 256], f32r, tag="phT", bufs=2, name=f"h2pT{b}")
            for srcT, dstT, nm in ((p2h, p2hT, "p"), (h2p, h2pT, "h")):
                ptr = attnr_ps(f"{nm}T{b}", (128, 512))
                for jc in range(2):
                    for ic in range(2):
                        k = jc * 2 + ic
                        nc.tensor.matmul(ptr[:, k * 128:(k + 1) * 128],
                                         lhsT=srcT[:, ic, jc * 128:(jc + 1) * 128],
                                         rhs=identr, is_transpose=True, start=(k == 0), stop=(k == 3))
                nc.vector.tensor_copy(dstT[:], ptr[:].rearrange("p (a b) -> p a b", a=2))

            if STAGE < 9:
                continue
            # ---- Y = cat @ Wc1_bot (token-major out, feature-major input) ----
            Yt = work.tile([128, 4, 512], f32r, tag="Y", name=f"Y{b}")
            for s in range(2):
                for tcn in range(2):
                    pm = mm512_ps(f"Y{b}_{s}{tcn}")
                    for kc in range(8):
                        nc.tensor.matmul(pm[:], lhsT=cmpin[:, kc, s * 256 + tcn * 128:s * 256 + (tcn + 1) * 128],
                                         rhs=wc1[:, 8 + kc, :], start=(kc == 0), stop=(kc == 7))
                    nc.vector.tensor_copy(Yt[:, s * 2 + tcn, :], pm[:])

            # ---- compare L1 (feature-major, both seqs) ----
            cmp1 = work.tile([128, 4, 512], bf16, tag="cmp1", name=f"cmp1_{b}")
            for nf in range(4):
                pm = mm512_ps(f"c1_{b}_nf{nf}")
                for kc in range(8):
                    nc.tensor.matmul(pm[:], lhsT=wc1[:, kc, nf * 128:(nf + 1) * 128],
                                     rhs=cmpin[:, kc, :], start=(kc == 0), stop=False)
                for tcn in range(2):
                    nc.tensor.matmul(pm[:, 0:256], lhsT=Yt[:, 2 + tcn, nf * 128:(nf + 1) * 128],
                                     rhs=p2hT[:, tcn, :], start=False, stop=False)
                for tcn in range(2):
                    nc.tensor.matmul(pm[:, 256:512], lhsT=Yt[:, tcn, nf * 128:(nf + 1) * 128],
                                     rhs=h2pT[:, tcn, :], start=False, stop=(tcn == 1))
                nc.scalar.activation(cmp1[:, nf, :], pm[:], AF.Relu, bias=bsb["b_c1"][:, nf:nf + 1])

            if STAGE < 10:
                continue
            # ---- compare L2 (token-major) + masked sum ----
            for s in range(2):
                cmp2 = work.tile([128, 2, 512], f32r, tag="cmp2", bufs=1, name=f"cmp2_{b}_{s}")
                for tcn in range(2):
                    pm = mm512_ps(f"c2_{b}_{s}{tcn}")
                    nc.tensor.matmul(pm[:], lhsT=ones, rhs=bc2row[:], start=True, stop=False)
                    for kc in range(4):
                        nc.tensor.matmul(pm[:], lhsT=cmp1[:, kc, s * 256 + tcn * 128:s * 256 + (tcn + 1) * 128],
                                         rhs=wc2[:, kc, :], start=False, stop=(kc == 3))
                    nc.scalar.activation(cmp2[:, tcn, :], pm[:], AF.Relu)
                pa = mm512_ps(f"sum{b}_{s}", (1, 512))
                for tcn in range(2):
                    nc.tensor.matmul(pa[:], lhsT=maskf[:, s, tcn:tcn + 1],
                                     rhs=cmp2[:, tcn, :], start=(tcn == 0), stop=(tcn == 1))
                srow = work.tile([1, 512], f32r, tag="sumrow", bufs=1, name=f"srow{b}_{s}")
                nc.vector.tensor_copy(srow[:], pa[:])
                # tiny SBUF->SBUF DMA moves the row to its s16 partition
                nc.sync.dma_start(s16[s * 32 + b:s * 32 + b + 1, :], srow[:])

            if b == 0 and dbg:
                def tap(name, src_ap):
                    if name in dbg:
                        nc.sync.dma_start(dbg[name].ap(), src_ap)
                tap("dbg_cmpin", cmpin[:].bitcast(f32))
                tap("dbg_q", qb[:].bitcast(f32))
                tap("dbg_att", att[:].bitcast(f32))
                tap("dbg_zm", zm[:].bitcast(f32))
                tap("dbg_p2h", p2h[:].bitcast(f32))
                tap("dbg_h2p", h2p[:].bitcast(f32))
                tap("dbg_Y", Yt[:].bitcast(f32))

        # ---------------- aggregate MLP (all items at once) ----------------
        # wg1/wg2 are only needed here; load them into retired cmpin slots
        # (same 16KB tag slot, allocations rotate past the item loop's).
        wg1 = work.tile([128, 8, D], f32r, tag="cmpin", bufs=2, name="wg1")
        nc.sync.dma_start(wg1[:], bass.AP(w_g1, 0, [[D, 128], [128 * D, 8], [1, D]]))
        wg2 = work.tile([128, 4, D], f32r, tag="cmpin", bufs=2, name="wg2")
        nc.scalar.dma_start(wg2[:], bass.AP(w_g2, 0, [[D, 128], [128 * D, 4], [1, D]]))

        run_agg = (nitems == BL) and STAGE >= 11
        if run_agg:
            # bias rows ride retired Yt slots; staging rows ride retired cmp1
            # slots (both dead after item 7) -> zero new SBUF bytes
            bg1row = work.tile([1, D], f32r, tag="Y", name="bg1row")
            nc.sync.dma_start(bg1row[:], bass.AP(bdram["b_g1"], 0, [[0, 1], [1, D]]).bitcast(f32r))
            bg2row = work.tile([1, D], f32r, tag="Y", name="bg2row")
            nc.sync.dma_start(bg2row[:], bass.AP(bdram["b_g2"], 0, [[0, 1], [1, D]]).bitcast(f32r))
            # transpose s16 [2*BL, 512] -> aggT [128, (s,kc4), BL] on the PE
            aggT = work.tile([128, 2, 4, BL], f32r, tag="aggT", name="aggT")
            for s in range(2):
                for kc4 in range(4):
                    pt = attnr_ps(f"s16T{s}{kc4}", (128, BL))
                    nc.tensor.matmul(pt[:], lhsT=s16[s * 32:s * 32 + BL, kc4 * 128:(kc4 + 1) * 128],
                                     rhs=identr[s * 32:s * 32 + BL, s * 32:s * 32 + BL],
                                     is_transpose=True, start=True, stop=True)
                    nc.vector.tensor_copy(aggT[:, s, kc4, :], pt[:])
            # L1: items on out partitions, full 512-wide free dim
            pm = mm512_ps("g1")
            nc.tensor.matmul(pm[0:BL, :], lhsT=ones[:, 0:BL], rhs=bg1row[:], start=True, stop=False)
            for kc in range(8):
                nc.tensor.matmul(pm[0:BL, :], lhsT=aggT[:, kc // 4, kc % 4, :],
                                 rhs=wg1[:, kc, :], start=False, stop=(kc == 7))
            agg1r = work.tile([BL, D], f32r, tag="cmp1", name="agg1r")
            nc.scalar.activation(agg1r[:], pm[0:BL, :], AF.Relu)
            agg1T = work.tile([128, 4, BL], f32r, tag="agg1", name="agg1T")
            for kc4 in range(4):
                pt = attnr_ps(f"a1T{kc4}", (128, BL))
                nc.tensor.matmul(pt[:], lhsT=agg1r[0:BL, kc4 * 128:(kc4 + 1) * 128],
                                 rhs=identr[0:BL, 0:BL], is_transpose=True, start=True, stop=True)
                nc.vector.tensor_copy(agg1T[:, kc4, :], pt[:])
            # L2: items on out partitions
            pm2 = mm512_ps("g2")
            nc.tensor.matmul(pm2[0:BL, :], lhsT=ones[:, 0:BL], rhs=bg2row[:], start=True, stop=False)
            for kc in range(4):
                nc.tensor.matmul(pm2[0:BL, :], lhsT=agg1T[:, kc, :],
                                 rhs=wg2[:, kc, :], start=False, stop=(kc == 3))
            agg2r = work.tile([BL, D], f32r, tag="cmp1", name="agg2r")
            nc.scalar.activation(agg2r[:], pm2[0:BL, :], AF.Relu)
            agg2T = work.tile([128, 4, BL], f32r, tag="agg2", name="agg2T")
            for kc4 in range(4):
                pt = attnr_ps(f"a2T{kc4}", (128, BL))
                nc.tensor.matmul(pt[:], lhsT=agg2r[0:BL, kc4 * 128:(kc4 + 1) * 128],
                                 rhs=identr[0:BL, 0:BL], is_transpose=True, start=True, stop=True)
                nc.vector.tensor_copy(agg2T[:, kc4, :], pt[:])
            po = attn_ps("po")
            for kc in range(4):
                nc.tensor.matmul(po[0:BL, 0:4], lhsT=agg2T[:, kc, :], rhs=wo[:, kc, :],
                                 start=(kc == 0), stop=(kc == 3))
            osb = work.tile([BL, OUT], f32, tag="cmp1", name="osb")
            nc.vector.tensor_copy(osb[:], po[0:BL, 0:OUT])
            nc.sync.dma_start(out_d.ap(), osb[:])

        ps.release()
        work.release()
        const.release()

    nc.compile()
    return nc


def _get_program(debug_taps=()):
    key = tuple(n for n, _ in debug_taps)
    if key not in _PROG_CACHE:
        _PROG_CACHE[key] = _build_program(debug_taps)
    return _PROG_CACHE[key]


def kernel(prem_input, hypo_input, embed_W, dist_W,
           Ws1, bs1, Ws2, bs2, Wa1, ba1, Wa2, ba2,
           Wc1, bc1, Wc2, bc2, Wg1, bg1, Wg2, bg2, Wo,
           _debug_taps=(), _trace=False, _tmpdir=None):
    from concourse.bass_utils import run_bass_kernel_spmd

    nc = _get_program(_debug_taps)

    f32 = np.float32
    # relative-distance bias [256,256]: dW[clip(j-i,-11,11)+11], diag -1e9
    pos = np.arange(L)
    idx = np.clip(pos[None, :] - pos[:, None], -MAX_DIST, MAX_DIST) + MAX_DIST
    bias_full = np.asarray(dist_W, f32).reshape(-1)[idx]
    np.fill_diagonal(bias_full, DIAG_VAL)
    common = {
        "emb": np.ascontiguousarray(embed_W, f32),
        "bias_m": np.ascontiguousarray(bias_full.reshape(2, 128, 256)),
        "w_s1": np.ascontiguousarray(Ws1, f32), "w_s2": np.ascontiguousarray(Ws2, f32),
        "w_a1": np.ascontiguousarray(Wa1, f32), "w_a2": np.ascontiguousarray(Wa2, f32),
        "w_c1": np.ascontiguousarray(Wc1, f32), "w_c2": np.ascontiguousarray(Wc2, f32),
        "w_g1": np.ascontiguousarray(Wg1, f32), "w_g2": np.ascontiguousarray(Wg2, f32),
        "w_o": np.ascontiguousarray(Wo, f32),
        "b_s1": np.ascontiguousarray(bs1, f32), "b_s2": np.ascontiguousarray(bs2, f32),
        "b_a1": np.ascontiguousarray(ba1, f32), "b_a2": np.ascontiguousarray(ba2, f32),
        "b_c1": np.ascontiguousarray(bc1, f32), "b_c2": np.ascontiguousarray(bc2, f32),
        "b_g1": np.ascontiguousarray(bg1, f32), "b_g2": np.ascontiguousarray(bg2, f32),
    }
    # the transport rounds away the low 12 bits of each 32-bit word; shift
    # indices into the exactly-preserved high bits and shift back on device
    prem = np.ascontiguousarray(np.asarray(prem_input).reshape(B, L).astype(np.int64) << 12).astype(np.int32)
    hypo = np.ascontiguousarray(np.asarray(hypo_input).reshape(B, L).astype(np.int64) << 12).astype(np.int32)

    prem_raw = np.asarray(prem_input).reshape(B, L)
    hypo_raw = np.asarray(hypo_input).reshape(B, L)
    in_maps = []
    for c in range(NCORES):
        sl = slice(c * BL, (c + 1) * BL)
        tokc = np.stack([prem[sl], hypo[sl]], axis=0)  # [2, BL, L]
        tr = np.stack([prem_raw[sl], hypo_raw[sl]], axis=1)  # [BL, 2, L]
        mf = (tr != 0).astype(f32).reshape(BL, 2, 2, 128)    # [b, s, tc, p]
        md = np.where(tr == 0, f32(MASK_VAL), f32(0.0)).astype(f32)  # [b, s, j]
        in_maps.append({"tok": np.ascontiguousarray(tokc),
                        "mf": np.ascontiguousarray(mf),
                        "md": np.ascontiguousarray(md), **common})

    kwargs = {}
    if _trace:
        kwargs.update(trace=True, tmpdir=_tmpdir)
    res = run_bass_kernel_spmd(nc, in_maps, core_ids=list(range(NCORES)), **kwargs)
    out = np.concatenate([r["out"] for r in res.results], axis=0)
    if _debug_taps or _trace:
        return out, res
    return out


# revision 80
# speedup vs baseline: 1.1862x; 1.1862x over previous
"""Trainium2 Bass kernel for nn_BernoulliDecompAttModel (decomposable attention NLI model).

Contract: kernel(**inputs) takes the FULL unsharded inputs (as produced by
setup_inputs()) and returns the FULL [64, 3] float32 output. Internally the
batch (64) is sharded 8-ways across 8 NeuronCores (pure data parallel, all
weights replicated); each core runs an identical Bass/Tile program on its 8
batch items.

Layout conventions inside the device program (per core):
  - Activations are kept "feature-major": [128 partitions = feature chunk,
    K/128 chunks, token axis on free dim]. Matmul contracts over partitions,
    so feature-major activations feed matmuls directly with weights stored
    natural [Kin (partitions x chunks), Nout].
  - prem and hypo (256 tokens each) are concatenated on the token axis for
    every shared-weight MLP -> N=512 matmuls.
  - All matmul operands are float32r (fp32 storage, 1 cycle/column on the PE
    vs 4 for plain fp32; ~2^-13 effective multiply rounding).
  - Attention: scores accumulate in PSUM; key masks are injected by a K=1
    matmul ones[1,128] (x) madd[1,256] accumulated into the same PSUM bank.
    The relative-distance bias matrix (incl. the score-diagonal -1e9) and the
    pad masks are precomputed on the host and DMA'd like weights.

Performance notes (964us -> ~515us on 8 trn2 cores):
  - PSUM tags are multi-buffered (mm512 x4 banks, attn x2, attnr x2) so the
    PE never waits for Scalar/Vector to drain an accumulation bank; keeping
    the PE continuously busy also lets it ramp from the 1.2 GHz mid p-state
    toward 2.4 GHz (the ramp needs ~3us of uninterrupted occupancy, so every
    avoided bubble also buys clock speed on the instructions that follow).
  - Token index DMAs + embedding gathers for all items are issued early and
    on separate trigger queues from the weight loads (weights on SP/sync,
    per-item loads on Activation/scalar, both in first-use order), so item-0
    matmul work starts ~15us in instead of ~70us.
  - cmpin/mid/qpq SBUF tags are double-buffered so item b+1's transposes and
    MLP fill the PE while item b is in its softmax chains; wg1/wg2 are loaded
    late into retired cmpin slots to make the footprint fit (224KB SBUF).
  - Four 128x128 PE transposes fill one 2KB PSUM bank and drain with a
    single strided DVE copy; softmaxes read the drained SBUF copy so the
    attention banks free as early as possible. The compare-L1 activations
    and wc2 run in bf16 (gpsimd casting DMA) to fund double-buffered
    embedding-gather tiles.
  - The per-item masked token sums collect in an SBUF [40,512] tile (via tiny
    SBUF->SBUF DMAs) and are PE-transposed for the aggregate MLP, avoiding a
    DRAM round-trip in the tail.
"""

import numpy as np
import os

B, L, V, E, D, OUT = 64, 256, 50000, 512, 512, 3
NCORES = 8
BL = B // NCORES            # batch items per core
MAX_DIST = 11
MASK_VAL = -30000.0         # padded-key additive mask (exp() underflows to 0)
DIAG_VAL = -1e9             # self-attention diagonal

_PROG_CACHE = {}


def _build_program(debug_taps=()):
    import concourse.bass as bass
    import concourse.bacc as bacc
    import concourse.mybir as mybir
    from concourse.tile import TileContext
    from concourse.masks import make_identity

    dt = mybir.dt
    f32, f32r, i32 = dt.float32, dt.float32r, dt.int32
    bf16 = dt.bfloat16
    AF = mybir.ActivationFunctionType
    ALU = mybir.AluOpType
    AX = mybir.AxisListType

    nc = bacc.Bacc("TRN2", target_bir_lowering=False, debug=True)

    # ---------------- DRAM I/O ----------------
    tok = nc.dram_tensor("tok", [2, BL, L], i32, kind="ExternalInput")
    emb = nc.dram_tensor("emb", [V, E], f32, kind="ExternalInput")
    # host-precomputed: bias_m[ic, p, j] = rel-bias[ic*128+p, j] (+ diag -1e9)
    bias_md = nc.dram_tensor("bias_m", [2, 128, 256], f32, kind="ExternalInput")
    # host-precomputed masks: mf[b, s, tc, p] = (tok != 0); md[b, s, j] = -3e4*(tok == 0)
    mf_d = nc.dram_tensor("mf", [BL, 2, 2, 128], f32r, kind="ExternalInput")
    md_d = nc.dram_tensor("md", [BL, 2, L], f32r, kind="ExternalInput")
    w_s1 = nc.dram_tensor("w_s1", [E, D], f32r, kind="ExternalInput")
    w_s2 = nc.dram_tensor("w_s2", [D, D], f32r, kind="ExternalInput")
    w_a1 = nc.dram_tensor("w_a1", [2 * E, D], f32r, kind="ExternalInput")
    w_a2 = nc.dram_tensor("w_a2", [D, D], f32r, kind="ExternalInput")
    w_c1 = nc.dram_tensor("w_c1", [4 * E, D], f32r, kind="ExternalInput")
    w_c2 = nc.dram_tensor("w_c2", [D, D], f32, kind="ExternalInput")
    w_g1 = nc.dram_tensor("w_g1", [2 * D, D], f32r, kind="ExternalInput")
    w_g2 = nc.dram_tensor("w_g2", [D, D], f32r, kind="ExternalInput")
    w_o = nc.dram_tensor("w_o", [D, OUT], f32r, kind="ExternalInput")
    bias_names = ["b_s1", "b_s2", "b_a1", "b_a2", "b_c1", "b_c2", "b_g1", "b_g2"]
    bdram = {n: nc.dram_tensor(n, [D], f32, kind="ExternalInput") for n in bias_names}

    out_d = nc.dram_tensor("out", [BL, OUT], f32, kind="ExternalOutput")

    dbg = {}
    for name, shape in debug_taps:
        dbg[name] = nc.dram_tensor(name, shape, f32, kind="ExternalOutput")

    with TileContext(nc) as tc:
        const = tc.alloc_tile_pool(name="const", bufs=1)
        work = tc.alloc_tile_pool(name="work", bufs=1)
        ps = tc.alloc_tile_pool(name="ps", bufs=1, space="PSUM")

        def mm512_ps(name, shape=(128, 512)):
            return ps.tile(list(shape), f32, space="PSUM", tag="mm512", bufs=4,
                           name=name)

        def attn_ps(name):
            return ps.tile([128, 256], f32, space="PSUM", tag="attn", bufs=2,
                           name=name)

        def attnr_ps(name, shape=(128, 256)):
            return ps.tile(list(shape), f32r, space="PSUM", tag="attnr", bufs=2,
                           name=name)

        # ---------------- small constants (no big DMA) ----------------
        ones32 = const.tile([1, 128], f32, name="ones32")
        nc.vector.memset(ones32[:], 1.0)
        onesr = const.tile([1, 128], f32r, name="onesr")
        nc.vector.tensor_copy(onesr[:], ones32[:])
        ones = onesr[:]

        ident32 = const.tile([128, 128], f32, name="ident32")
        make_identity(nc, ident32[:])
        identr_t = const.tile([128, 128], f32r, name="identr_t")
        nc.vector.tensor_copy(identr_t[:], ident32[:])
        identr = identr_t[:]

        # Weights all on the sync queue in first-use order; per-item DMAs on
        # the scalar queue. Self-MLP weights first: item 0 needs them ~15us in.
        def load_w(dram, K, N, name, eng):
            t = const.tile([128, K // 128, N], f32r, name=name)
            eng.dma_start(t[:], bass.AP(dram, 0, [[N, 128], [128 * N, K // 128], [1, N]]))
            return t

        bsb = {}

        def load_b(n):
            t = const.tile([128, 4], f32, name=f"sb_{n}")
            nc.sync.dma_start(t[:], bass.AP(bdram[n], 0, [[1, 128], [128, 4]]))
            bsb[n] = t

        ws1 = load_w(w_s1, E, D, "ws1", nc.sync)
        ws2 = load_w(w_s2, D, D, "ws2", nc.sync)
        load_b("b_s1")
        load_b("b_s2")
        # relative-distance bias matrix (host-precomputed, incl. diag -1e9)
        bias_sb = const.tile([128, 2, 256], f32, name="bias_sb")
        nc.sync.dma_start(bias_sb[:], bass.AP(bias_md, 0, [[256, 128], [128 * 256, 2], [1, 256]]))

        # ---------------- per-item indices / masks / gathers ----------------
        # Issued before the remaining weight loads so early item PE work can
        # start while weights stream in.
        nitems = int(os.environ.get('KITEMS', BL))
        STAGE = int(os.environ.get('KSTAGE', 99))

        it_all, maskf_all, madd_all, xembf_all = {}, {}, {}, {}
        for b in range(nitems):
            it = work.tile([128, 2, 2], i32, tag="it", bufs=4, name=f"it{b}")
            for s in range(2):
                nc.scalar.dma_start(it[:, s, :], bass.AP(tok, b * L + s * BL * L, [[1, 128], [128, 2]]))
            nc.vector.tensor_scalar(it[:], it[:], 12, None, op0=ALU.arith_shift_right)
            maskf = work.tile([128, 2, 2], f32r, tag="maskf", bufs=8, name=f"maskf{b}")
            nc.scalar.dma_start(maskf[:].rearrange("p a b -> p (a b)"),
                                bass.AP(mf_d, b * 512, [[1, 128], [128, 4]]))
            madd = work.tile([1, 2, L], f32r, tag="madd", bufs=2, name=f"madd{b}")
            nc.scalar.dma_start(madd[:], bass.AP(md_d, b * 2 * L, [[0, 1], [L, 2], [1, L]]))

            # embedding gather (token-major), straight into f32r tiles (DMA
            # outputs need no explicit f32r rounding op)
            xembf = [work.tile([128, E], f32r, tag=f"xembf{i}", bufs=2, name=f"xembf{b}_{i}")
                     for i in range(4)]
            for s in range(2):
                for tcn in range(2):
                    i = s * 2 + tcn
                    nc.gpsimd.indirect_dma_start(
                        out=xembf[i][:, :], out_offset=None, in_=emb.ap().bitcast(f32r),
                        in_offset=bass.IndirectOffsetOnAxis(ap=it[:, s, tcn:tcn + 1], axis=0))
            it_all[b], maskf_all[b], madd_all[b], xembf_all[b] = it, maskf, madd, xembf

        if "dbg_it" in dbg:
            itf = work.tile([128, 4], f32, tag="dbgitf", name="dbgitf")
            nc.vector.tensor_copy(itf[:], it_all[0][:].rearrange("p a b -> p (a b)"))
            nc.sync.dma_start(dbg["dbg_it"].ap(), itf[:])
            embrow = work.tile([2, 512], f32, tag="dbgemb", name="dbgemb")
            nc.sync.dma_start(embrow[0:1, :], bass.AP(emb, 7402 * 512, [[0, 1], [1, 512]]))
            nc.sync.dma_start(embrow[1:2, :], bass.AP(emb, 8192 * 512, [[0, 1], [1, 512]]))
            nc.sync.dma_start(dbg["dbg_embrow"].ap(), embrow[:])
        if "dbg_xemb" in dbg:
            for i in range(4):
                nc.sync.dma_start(bass.AP(dbg["dbg_xemb"], i * 512, [[4 * 512, 128], [1, 512]]),
                                  xembf_all[0][i][:, :].bitcast(f32))

        # ---------------- remaining weights (first-use order) ----
        wa1 = load_w(w_a1, 2 * E, D, "wa1", nc.sync)
        wa2 = load_w(w_a2, D, D, "wa2", nc.sync)
        load_b("b_a1")
        load_b("b_a2")
        wc1 = load_w(w_c1, 4 * E, D, "wc1", nc.sync)
        wc2 = const.tile([128, 4, D], bf16, name="wc2")
        nc.gpsimd.dma_start(wc2[:], bass.AP(w_c2, 0, [[D, 128], [128 * D, 4], [1, D]]))
        load_b("b_c1")
        load_b("b_c2")
        bc2row = const.tile([1, D], f32r, name="bc2row")
        nc.sync.dma_start(bc2row[:], bass.AP(bdram["b_c2"], 0, [[0, 1], [1, D]]).bitcast(f32r))
        load_b("b_g1")
        load_b("b_g2")
        wo = const.tile([128, 4, 4], f32r, name="wo")
        nc.vector.memset(wo[:].bitcast(f32), 0.0)
        nc.sync.dma_start(wo[:, :, 0:OUT], bass.AP(w_o, 0, [[OUT, 128], [128 * OUT, 4], [1, OUT]]))

        # masked per-item sums collect here; prem rows at partitions 0..7,
        # hypo rows at 32..39 (matmul lhsT base partition must be 0/32/64)
        s16 = work.tile([32 + BL, 512], f32r, tag="s16", name="s16")

        # ---------------- per-item pipeline ----------------
        def softmax_rows(src_ap, dst_ap, tag_suffix):
            """row softmax: src_ap [128,256] (SBUF or PSUM, f32 view) -> dst_ap f32r"""
            negmax = work.tile([128, 1], f32, tag="negmax", bufs=4, name=f"negmax{tag_suffix}")
            nc.vector.reduce_max(negmax[:], src_ap, axis=AX.X, negate=True)
            esum = work.tile([128, 1], f32, tag="esum", bufs=4, name=f"esum{tag_suffix}")
            nc.scalar.activation(dst_ap, src_ap, AF.Exp, bias=negmax[:], scale=1.0,
                                 accum_out=esum[:])
            rec = work.tile([128, 1], f32, tag="rec", bufs=4, name=f"rec{tag_suffix}")
            nc.vector.reciprocal(rec[:], esum[:])
            nc.vector.tensor_scalar(dst_ap, dst_ap, rec[:, 0:1], None, op0=ALU.mult)

        for b in range(nitems):
            it, maskf, madd, xembf = it_all[b], maskf_all[b], madd_all[b], xembf_all[b]
            xemb = [t[:] for t in xembf]

            if STAGE < 2:
                continue
            # ---- x transposes -> cmpin kc 0..3 (feature-major cat, both seqs) ----
            # four 128x128 transposes fill one 2KB PSUM bank -> one strided drain
            cmpin = work.tile([128, 8, 512], f32r, tag="cmpin", bufs=2, name=f"cmpin{b}")
            for s in range(2):
                for tcn in range(2):
                    ptr = attnr_ps(f"xT{b}_{s}{tcn}0", (128, 512))
                    for k in range(4):
                        nc.tensor.matmul(ptr[:, k * 128:(k + 1) * 128],
                                         lhsT=xemb[s * 2 + tcn][:, k * 128:(k + 1) * 128],
                                         rhs=identr, is_transpose=True, start=(k == 0), stop=(k == 3))
                    nc.vector.tensor_copy(
                        cmpin[:, 0:4, s * 256 + tcn * 128:s * 256 + (tcn + 1) * 128],
                        ptr[:].rearrange("p (a b) -> p a b", a=4))

            if STAGE < 3:
                continue
            # ---- self MLP (shared weights, both seqs: N=512) ----
            def mlp_fm(src, src_kcs, w, bias_t, dst, name):
                """feature-major MLP layer: dst[:,nf,:] = relu(w.T @ src + bias)

                The last chunk's drain runs on DVE (bias-add then max-0 in one
                tensor_scalar) so a consumer needing all 4 chunks isn't
                serialized behind Scalar's queue."""
                nkc = len(src_kcs)
                for nf in range(4):
                    pm = mm512_ps(f"{name}_nf{nf}")
                    for i, kc in enumerate(src_kcs):
                        nc.tensor.matmul(pm[:], lhsT=w[:, kc, nf * 128:(nf + 1) * 128],
                                         rhs=src[:, kc, :], start=(i == 0), stop=(i == nkc - 1))
                    nc.scalar.activation(dst[:, nf, :], pm[:], AF.Relu, bias=bias_t[:, nf:nf + 1])

            hmid = work.tile([128, 4, 512], f32r, tag="mid", bufs=2, name=f"h1_{b}")
            mlp_fm(cmpin, range(4), ws1, bsb["b_s1"], hmid, f"sm1_{b}")
            qb = work.tile([128, 4, 512], f32r, tag="qpq", bufs=2, name=f"q_{b}")
            mlp_fm(hmid, range(4), ws2, bsb["b_s2"], qb, f"sm2_{b}")

            if STAGE < 4:
                continue
            # ---- self attention per sequence ----
            att = work.tile([128, 4, 256], f32r, tag="att", name=f"att{b}")
            for s in range(2):
                for ic in range(2):
                    pS = attn_ps(f"S{b}_{s}{ic}")
                    nc.tensor.matmul(pS[:], lhsT=ones, rhs=madd[0:1, s, :],
                                     start=True, stop=False)
                    for kc in range(4):
                        nc.tensor.matmul(pS[:], lhsT=qb[:, kc, s * 256 + ic * 128:s * 256 + (ic + 1) * 128],
                                         rhs=qb[:, kc, s * 256:(s + 1) * 256],
                                         start=False, stop=(kc == 3))
                    sm = work.tile([128, 256], f32, tag="sm", bufs=2, name=f"sm{b}_{s}{ic}")
                    nc.vector.tensor_tensor(sm[:], pS[:], bias_sb[:, ic, :], op=ALU.add)
                    softmax_rows(sm[:], att[:, s * 2 + ic, :], f"_att{b}_{s}{ic}")

            if STAGE < 5:
                continue
            attT = work.tile([128, 4, 256], f32r, tag="attT", name=f"attT{b}")
            for s in range(2):
                ptr = attnr_ps(f"attT{b}_{s}", (128, 512))
                for jc in range(2):
                    for ic in range(2):
                        k = jc * 2 + ic
                        nc.tensor.matmul(ptr[:, k * 128:(k + 1) * 128],
                                         lhsT=att[:, s * 2 + ic, jc * 128:(jc + 1) * 128],
                                         rhs=identr, is_transpose=True, start=(k == 0), stop=(k == 3))
                nc.vector.tensor_copy(attT[:, s * 2:s * 2 + 2, :],
                                      ptr[:].rearrange("p (a b) -> p a b", a=2))

            # ---- ctx feature-major -> cmpin[:, 4+dc, :] ----
            for s in range(2):
                for dc in range(4):
                    pm = attn_ps(f"ctxT{b}_{s}{dc}")
                    for jc in range(2):
                        nc.tensor.matmul(pm[:], lhsT=xemb[s * 2 + jc][:, dc * 128:(dc + 1) * 128],
                                         rhs=attT[:, s * 2 + jc, :], start=(jc == 0), stop=(jc == 1))
                    nc.vector.tensor_copy(cmpin[:, 4 + dc, s * 256:(s + 1) * 256], pm[:])

            if STAGE < 6:
                continue
            # ---- inter MLP (input = cmpin kc 0..7, K=1024) ----
            hmid2 = work.tile([128, 4, 512], f32r, tag="mid", bufs=2, name=f"h2_{b}")
            mlp_fm(cmpin, range(8), wa1, bsb["b_a1"], hmid2, f"im1_{b}")
            qb2 = work.tile([128, 4, 512], f32r, tag="qpq", bufs=2, name=f"q2_{b}")
            mlp_fm(hmid2, range(4), wa2, bsb["b_a2"], qb2, f"im2_{b}")  # qb2 = [pq | hk]

            if STAGE < 7:
                continue
            # ---- inter attention z = pq @ hk^T ----
            zm = work.tile([128, 2, 256], f32r, tag="zm", name=f"zm{b}")
            p2h = work.tile([128, 2, 256], f32r, tag="ph", bufs=2, name=f"p2h{b}")
            for ic in range(2):
                pz = attn_ps(f"z{b}_{ic}")
                nc.tensor.matmul(pz[:], lhsT=ones, rhs=madd[0:1, 1, :], start=True, stop=False)
                for kc in range(4):
                    nc.tensor.matmul(pz[:], lhsT=qb2[:, kc, ic * 128:(ic + 1) * 128],
                                     rhs=qb2[:, kc, 256:512], start=False, stop=(kc == 3))
                nc.vector.tensor_copy(zm[:, ic, :], pz[:])
                softmax_rows(zm[:, ic, :].bitcast(f32), p2h[:, ic, :], f"_p2h{b}_{ic}")

            h2p = work.tile([128, 2, 256], f32r, tag="ph", bufs=2, name=f"h2p{b}")
            for jc in range(2):
                pzT = attnr_ps(f"zT{b}_{jc}")
                nc.tensor.matmul(pzT[:].bitcast(f32), lhsT=ones, rhs=madd[0:1, 0, :].bitcast(f32r),
                                 start=True, stop=False)
                for ic in range(2):
                    nc.tensor.matmul(pzT[:, ic * 128:(ic + 1) * 128],
                                     lhsT=zm[:, ic, jc * 128:(jc + 1) * 128],
                                     rhs=identr, is_transpose=True, start=False, stop=(ic == 1))
                softmax_rows(pzT[:].bitcast(f32), h2p[:, jc, :], f"_h2p{b}_{jc}")

            if STAGE < 8:
                continue
            p2hT = work.tile([128, 2, 256], f32r, tag="phT", bufs=2, name=f"p2hT{b}")
            h2pT = work.tile([128, 2, 256], f32r, tag="phT", bufs=2, name=f"h2pT{b}")
            for srcT, dstT, nm in ((p2h, p2hT, "p"), (h2p, h2pT, "h")):
                ptr = attnr_ps(f"{nm}T{b}", (128, 512))
                for jc in range(2):
                    for ic in range(2):
                        k = jc * 2 + ic
                        nc.tensor.matmul(ptr[:, k * 128:(k + 1) * 128],
                                         lhsT=srcT[:, ic, jc * 128:(jc + 1) * 128],
                                         rhs=identr, is_transpose=True, start=(k == 0), stop=(k == 3))
                nc.vector.tensor_copy(dstT[:], ptr[:].rearrange("p (a b) -> p a b", a=2))

            if STAGE < 9:
                continue
            # ---- Y = cat @ Wc1_bot (token-major out, feature-major input) ----
            Yt = work.tile([128, 4, 512], f32r, tag="Y", name=f"Y{b}")
            for s in range(2):
                for tcn in range(2):
                    pm = mm512_ps(f"Y{b}_{s}{tcn}")
                    for kc in range(8):
                        nc.tensor.matmul(pm[:], lhsT=cmpin[:, kc, s * 256 + tcn * 128:s * 256 + (tcn + 1) * 128],
                                         rhs=wc1[:, 8 + kc, :], start=(kc == 0), stop=(kc == 7))
                    nc.vector.tensor_copy(Yt[:, s * 2 + tcn, :], pm[:])

            # ---- compare L1 (feature-major, both seqs) ----
            cmp1 = work.tile([128, 4, 512], bf16, tag="cmp1", name=f"cmp1_{b}")
            for nf in range(4):
                pm = mm512_ps(f"c1_{b}_nf{nf}")
                for kc in range(8):
                    nc.tensor.matmul(pm[:], lhsT=wc1[:, kc, nf * 128:(nf + 1) * 128],
                                     rhs=cmpin[:, kc, :], start=(kc == 0), stop=False)
                for tcn in range(2):
                    nc.tensor.matmul(pm[:, 0:256], lhsT=Yt[:, 2 + tcn, nf * 128:(nf + 1) * 128],
                                     rhs=p2hT[:, tcn, :], start=False, stop=False)
                for tcn in range(2):
                    nc.tensor.matmul(pm[:, 256:512], lhsT=Yt[:, tcn, nf * 128:(nf + 1) * 128],
                                     rhs=h2pT[:, tcn, :], start=False, stop=(tcn == 1))
                nc.scalar.activation(cmp1[:, nf, :], pm[:], AF.Relu, bias=bsb["b_c1"][:, nf:nf + 1])

            if STAGE < 10:
                continue
            # ---- compare L2 (token-major) + masked sum ----
            for s in range(2):
                cmp2 = work.tile([128, 2, 512], f32r, tag="cmp2", bufs=1, name=f"cmp2_{b}_{s}")
                for tcn in range(2):
                    pm = mm512_ps(f"c2_{b}_{s}{tcn}")
                    nc.tensor.matmul(pm[:], lhsT=ones, rhs=bc2row[:], start=True, stop=False)
                    for kc in range(4):
                        nc.tensor.matmul(pm[:], lhsT=cmp1[:, kc, s * 256 + tcn * 128:s * 256 + (tcn + 1) * 128],
                                         rhs=wc2[:, kc, :], start=False, stop=(kc == 3))
                    nc.scalar.activation(cmp2[:, tcn, :], pm[:], AF.Relu)
                pa = mm512_ps(f"sum{b}_{s}", (1, 512))
                for tcn in range(2):
                    nc.tensor.matmul(pa[:], lhsT=maskf[:, s, tcn:tcn + 1],
                                     rhs=cmp2[:, tcn, :], start=(tcn == 0), stop=(tcn == 1))
                srow = work.tile([1, 512], f32r, tag="sumrow", bufs=1, name=f"srow{b}_{s}")
                nc.vector.tensor_copy(srow[:], pa[:])
                # tiny SBUF->SBUF DMA moves the row to its s16 partition
                nc.sync.dma_start(s16[s * 32 + b:s * 32 + b + 1, :], srow[:])

            if b == 0 and dbg:
                def tap(name, src_ap):
                    if name in dbg:
                        nc.sync.dma_start(dbg[name].ap(), src_ap)
                tap("dbg_cmpin", cmpin[:].bitcast(f32))
                tap("dbg_q", qb[:].bitcast(f32))
                tap("dbg_att", att[:].bitcast(f32))
                tap("dbg_zm", zm[:].bitcast(f32))
                tap("dbg_p2h", p2h[:].bitcast(f32))
                tap("dbg_h2p", h2p[:].bitcast(f32))
                tap("dbg_Y", Yt[:].bitcast(f32))

        # ---------------- aggregate MLP (all items at once) ----------------
        # wg1/wg2 are only needed here; load them into retired cmpin slots
        # (same 16KB tag slot, allocations rotate past the item loop's).
        wg1 = work.tile([128, 8, D], f32r, tag="cmpin", bufs=2, name="wg1")
        nc.sync.dma_start(wg1[:], bass.AP(w_g1, 0, [[D, 128], [128 * D, 8], [1, D]]))
        wg2 = work.tile([128, 4, D], f32r, tag="cmpin", bufs=2, name="wg2")
        nc.scalar.dma_start(wg2[:], bass.AP(w_g2, 0, [[D, 128], [128 * D, 4], [1, D]]))

        run_agg = (nitems == BL) and STAGE >= 11
        if run_agg:
            # bias rows ride retired Yt slots; staging rows ride retired cmp1
            # slots (both dead after item 7) -> zero new SBUF bytes
            bg1row = work.tile([1, D], f32r, tag="Y", name="bg1row")
            nc.sync.dma_start(bg1row[:], bass.AP(bdram["b_g1"], 0, [[0, 1], [1, D]]).bitcast(f32r))
            bg2row = work.tile([1, D], f32r, tag="Y", name="bg2row")
            nc.sync.dma_start(bg2row[:], bass.AP(bdram["b_g2"], 0, [[0, 1], [1, D]]).bitcast(f32r))
            # transpose s16 [2*BL, 512] -> aggT [128, (s,kc4), BL] on the PE
            aggT = work.tile([128, 2, 4, BL], f32r, tag="aggT", name="aggT")
            for s in range(2):
                for kc4 in range(4):
                    pt = attnr_ps(f"s16T{s}{kc4}", (128, BL))
                    nc.tensor.matmul(pt[:], lhsT=s16[s * 32:s * 32 + BL, kc4 * 128:(kc4 + 1) * 128],
                                     rhs=identr[s * 32:s * 32 + BL, s * 32:s * 32 + BL],
                                     is_transpose=True, start=True, stop=True)
                    nc.vector.tensor_copy(aggT[:, s, kc4, :], pt[:])
            # L1: items on out partitions, full 512-wide free dim
            pm = mm512_ps("g1")
            nc.tensor.matmul(pm[0:BL, :], lhsT=ones[:, 0:BL], rhs=bg1row[:], start=True, stop=False)
            for kc in range(8):
                nc.tensor.matmul(pm[0:BL, :], lhsT=aggT[:, kc // 4, kc % 4, :],
                                 rhs=wg1[:, kc, :], start=False, stop=(kc == 7))
            agg1r = work.tile([BL, D], f32r, tag="cmp1", name="agg1r")
            nc.scalar.activation(agg1r[:], pm[0:BL, :], AF.Relu)
            agg1T = work.tile([128, 4, BL], f32r, tag="agg1", name="agg1T")
            for kc4 in range(4):
                pt = attnr_ps(f"a1T{kc4}", (128, BL))
                nc.tensor.matmul(pt[:], lhsT=agg1r[0:BL, kc4 * 128:(kc4 + 1) * 128],
                                 rhs=identr[0:BL, 0:BL], is_transpose=True, start=True, stop=True)
                nc.vector.tensor_copy(agg1T[:, kc4, :], pt[:])
            # L2: items on out partitions
            pm2 = mm512_ps("g2")
            nc.tensor.matmul(pm2[0:BL, :], lhsT=ones[:, 0:BL], rhs=bg2row[:], start=True, stop=False)
            for kc in range(4):
                nc.tensor.matmul(pm2[0:BL, :], lhsT=agg1T[:, kc, :],
                                 rhs=wg2[:, kc, :], start=False, stop=(kc == 3))
            agg2r = work.tile([BL, D], f32r, tag="cmp1", name="agg2r")
            nc.scalar.activation(agg2r[:], pm2[0:BL, :], AF.Relu)
            agg2T = work.tile([128, 4, BL], f32r, tag="agg2", name="agg2T")
            for kc4 in range(4):
                pt = attnr_ps(f"a2T{kc4}", (128, BL))
                nc.tensor.matmul(pt[:], lhsT=agg2r[0:BL, kc4 * 128:(kc4 + 1) * 128],
                                 rhs=identr[0:BL, 0:BL], is_transpose=True, start=True, stop=True)
                nc.vector.tensor_copy(agg2T[:, kc4, :], pt[:])
            po = attn_ps("po")
            for kc in range(4):
                nc.tensor.matmul(po[0:BL, 0:4], lhsT=agg2T[:, kc, :], rhs=wo[:, kc, :],
                                 start=(kc == 0), stop=(kc == 3))
            osb = work.tile([BL, OUT], f32, tag="cmp1", name="osb")
            nc.vector.tensor_copy(osb[:], po[0:BL, 0:OUT])
            nc.sync.dma_start(out_d.ap(), osb[:])

        ps.release()
        work.release()
        const.release()

    nc.compile()
    return nc


def _get_program(debug_taps=()):
    key = tuple(n for n, _ in debug_taps)
    if key not in _PROG_CACHE:
        _PROG_CACHE[key] = _build_program(debug_taps)
    return _PROG_CACHE[key]


def kernel(prem_input, hypo_input, embed_W, dist_W,
           Ws1, bs1, Ws2, bs2, Wa1, ba1, Wa2, ba2,
           Wc1, bc1, Wc2, bc2, Wg1, bg1, Wg2, bg2, Wo,
           _debug_taps=(), _trace=False, _tmpdir=None):
    from concourse.bass_utils import run_bass_kernel_spmd

    nc = _get_program(_debug_taps)

    f32 = np.float32
    # relative-distance bias [256,256]: dW[clip(j-i,-11,11)+11], diag -1e9
    pos = np.arange(L)
    idx = np.clip(pos[None, :] - pos[:, None], -MAX_DIST, MAX_DIST) + MAX_DIST
    bias_full = np.asarray(dist_W, f32).reshape(-1)[idx]
    np.fill_diagonal(bias_full, DIAG_VAL)
    common = {
        "emb": np.ascontiguousarray(embed_W, f32),
        "bias_m": np.ascontiguousarray(bias_full.reshape(2, 128, 256)),
        "w_s1": np.ascontiguousarray(Ws1, f32), "w_s2": np.ascontiguousarray(Ws2, f32),
        "w_a1": np.ascontiguousarray(Wa1, f32), "w_a2": np.ascontiguousarray(Wa2, f32),
        "w_c1": np.ascontiguousarray(Wc1, f32), "w_c2": np.ascontiguousarray(Wc2, f32),
        "w_g1": np.ascontiguousarray(Wg1, f32), "w_g2": np.ascontiguousarray(Wg2, f32),
        "w_o": np.ascontiguousarray(Wo, f32),
        "b_s1": np.ascontiguousarray(bs1, f32), "b_s2": np.ascontiguousarray(bs2, f32),
        "b_a1": np.ascontiguousarray(ba1, f32), "b_a2": np.ascontiguousarray(ba2, f32),
        "b_c1": np.ascontiguousarray(bc1, f32), "b_c2": np.ascontiguousarray(bc2, f32),
        "b_g1": np.ascontiguousarray(bg1, f32), "b_g2": np.ascontiguousarray(bg2, f32),
    }
    # the transport rounds away the low 12 bits of each 32-bit word; shift
    # indices into the exactly-preserved high bits and shift back on device
    prem = np.ascontiguousarray(np.asarray(prem_input).reshape(B, L).astype(np.int64) << 12).astype(np.int32)
    hypo = np.ascontiguousarray(np.asarray(hypo_input).reshape(B, L).astype(np.int64) << 12).astype(np.int32)

    prem_raw = np.asarray(prem_input).reshape(B, L)
    hypo_raw = np.asarray(hypo_input).reshape(B, L)
    in_maps = []
    for c in range(NCORES):
        sl = slice(c * BL, (c + 1) * BL)
        tokc = np.stack([prem[sl], hypo[sl]], axis=0)  # [2, BL, L]
        tr = np.stack([prem_raw[sl], hypo_raw[sl]], axis=1)  # [BL, 2, L]
        mf = (tr != 0).astype(f32).reshape(BL, 2, 2, 128)    # [b, s, tc, p]
        md = np.where(tr == 0, f32(MASK_VAL), f32(0.0)).astype(f32)  # [b, s, j]
        in_maps.append({"tok": np.ascontiguousarray(tokc),
                        "mf": np.ascontiguousarray(mf),
                        "md": np.ascontiguousarray(md), **common})

    kwargs = {}
    if _trace:
        kwargs.update(trace=True, tmpdir=_tmpdir)
    res = run_bass_kernel_spmd(nc, in_maps, core_ids=list(range(NCORES)), **kwargs)
    out = np.concatenate([r["out"] for r in res.results], axis=0)
    if _debug_taps or _trace:
        return out, res
    return out
